# revision 17
# baseline (speedup 1.0000x reference)
"""CustomMaxAbsPool2d Trainium2 Bass kernel.

Reference semantics (K=S=2, NCHW, VALID padding):
    abs_x = |x|; max_abs = maxpool(abs_x); up = nearest-upsample(max_abs)
    mask = (abs_x == up); out = maxpool(x * mask)

Per 2x2 window with p = max(v), q = min(v):
    p >= -q  <=>  p >= max|v|  <=>  the window max-abs element is positive,
    and then the masked maxpool returns p. Otherwise every max-abs element
    is negative, masked-out elements contribute 0, and the pool returns 0.
So out = p * (p >= -q). (The measure-zero exceptions -- an all-equal-
negative window, or p == -q exactly -- cannot occur with continuous
random input; validated bit-exact against the reference on hardware.)

Implementation: one fused custom DVE op per 16-row tile over paged
streams [P, S, N=2] (page = one output pixel; the two in-page elements
are the window's two columns; Src0/Src1 = the window's even/odd input
rows, loaded as separate row-parity DMA streams):

    m  = max(Src0, Src1)        vertical max
    nm = -min(Src0, Src1)       vertical max of negated values
    p  = page-scan MAX of m     (reset at each page boundary)
    nq = page-scan MAX of nm
    z  = p * (p >= nq)          valid at the 2nd element of each page

The per-page reset patches the lowered FSM's step state: at each
SUB_DIM_DONE the scan stages compute op(init, expr) instead of
op(prev, expr) -- the same override the seed state uses, applied to the
page-boundary element. A DVE copy extracts z[:, :, 1] (the valid lanes)
into a 4-tile store buffer; W=256 is even, so flattening rows x cols
keeps column pairs page-aligned and one [P, 1024-page] stream covers a
whole tile.

Sharding: pure data parallel over batch. Core k takes x[2k:2k+2] =>
128 images of 256x256, one image per SBUF partition.

Per-core engine budget (cost model): DMA ~116us (40MB @ ~358GB/s HBM
roofline), DVE ~40us, ACT ~20us -- memory-bound as targeted.
"""

from contextlib import ExitStack

import numpy as np

import concourse.bass as bass
import concourse.dve_ops as _dve_ops
import concourse.dve_spec as _ds
import concourse.tile as tile
from concourse import bacc, mybir
from concourse.bass_utils import run_bass_kernel_spmd
from concourse.dve_spec import AluOp, Spec, Src0, Src1, Zero, lower, maxx, minn, scan
from concourse.dve_uop import DveOpSpec

N, C, H, W = 16, 64, 256, 256
NCORES = 8
NB = N // NCORES
P = NB * C                # 128 images per core -> SBUF partitions
OH, OW = H // 2, W // 2
R = 16                    # input rows per tile
RO = R // 2
NT = H // R

F32 = mybir.dt.float32
I32 = mybir.dt.int32
AF = mybir.ActivationFunctionType

# --- custom DVE op registration -------------------------------------------

_orig_scan_overrides = _ds._scan_overrides


def _scan_overrides_page_reset(scans, node_stage):
    """Plain scans inside a subdim spec re-seed (op(init, expr)) at each
    SUB_DIM_DONE instead of carrying the fold across page boundaries."""
    seed, step = _orig_scan_overrides(scans, node_stage)
    for s in scans:
        if s._subdim_step is None:
            step[node_stage[s]] = _ds._Stage(s.op, _ds._scan_init(s), s.expr)
    return seed, step


def _maxabs_ref(in0, in1, s0, s1, imm2):
    v = np.stack([in0, in1]).astype(np.float32)
    m = v.max(axis=0)
    nm = (-v).max(axis=0)
    pp = np.maximum.accumulate(m, axis=-1)
    nn = np.maximum.accumulate(nm, axis=-1)
    return (pp * (pp >= nn)).astype(np.float32)


def _register_op():
    for op in _dve_ops.OPS:
        if op.name == "MAXABS_POOL_ANT":
            return op
    _ds._scan_overrides = _scan_overrides_page_reset
    m = maxx(Src0, Src1)
    nm = Zero - minn(Src0, Src1)
    p = scan(AluOp.MAX, m)
    nq = scan(AluOp.MAX, nm)
    spec = Spec(body=p * (p >= nq), reference=_maxabs_ref)
    row = _dve_ops._CUSTOM_DVE_ROW_BASE + len(_dve_ops.OPS)
    shas = {
        ver: DveOpSpec(
            name="MAXABS_POOL_ANT", opcode=row, uops=lower(spec, ver=ver),
            rd1_en=True,
        ).sha(ver)
        for ver in ("v3", "v4")
    }
    op = _dve_ops.DveOp("MAXABS_POOL_ANT", spec, subdim=True, uops_sha=shas)
    _dve_ops.OPS.append(op)
    _dve_ops._SUB_OPCODE_FOR_NAME[op.name] = row
    _dve_ops.CUSTOM_DVE_SPECS[op.name] = spec
    return op


MAXABS_POOL = _register_op()

# --- kernel ----------------------------------------------------------------


def build_nc() -> bass.Bass:
    nc = bacc.Bacc("TRN2", debug=False, num_swdge_queues=4)
    x = nc.dram_tensor("x", [P, H, W], F32, kind="ExternalInput").ap()
    y = nc.dram_tensor("y", [P, OH, OW], F32, kind="ExternalOutput").ap()
    xrows = x.rearrange("p (r two) w -> p r two w", two=2)   # row parity view

    # Tiling: 15 big tiles of 16 input rows, then 4 mini tiles of 4 rows.
    # The minis shorten the end-of-kernel critical chain (last input DMA ->
    # DVE -> copy -> writeback): only a 4-row compute hangs off the final
    # input transfer instead of a 16-row one.
    MR = 4
    NMINI = R // MR
    tiles = [(t * R, R) for t in range(NT - 1)]
    for j in range(NMINI):
        tiles.append(((NT - 1) * R + j * MR, MR))
    # writeback groups of big tiles (ncn must be pow2 -> sizes 4,4,4,2,1);
    # the minis share one writeback prepared early + fired by trigger.
    groups = [(0, 4, 0), (4, 4, 1), (8, 4, 2), (12, 2, 0), (14, 1, 1)]

    # Output stores go through gpsimd kv_writeback instead of plain DMA:
    # d_head=128 (one "head row" per partition), n_ctx = the whole 16K-elem
    # per-partition output slab, ctx_idx[b] = elem offset of batch block b.
    # Its SWDGE descriptors are striped 16-per-DMA-engine, so a store holds
    # the DMA engines ~14x shorter than the equivalent bulk DMACopy. The
    # tile framework serializes SWDGE DMAs per queue (each writeback waits
    # for the previous completion on its queue), so queues rotate.
    mini_base = (NT - 1) * R // 2 * OW
    idx_np_list = [t0 * RO * OW for t0, _, _ in groups] + [mini_base]
    NG = len(groups)
    idx_np = np.broadcast_to(
        np.array(idx_np_list, dtype=np.int32)[None, :], (P, NG + 1)
    ).copy()
    idxs_dram = nc.inline_tensor(idx_np, name="ctx_idxs").ap()
    # [batch, d_head_inner=128, d_head_outer=1, n_ctx] view of y
    yv = y.rearrange("(a p) (b h) w -> a p b (h w)", a=1, b=1)

    wb_sem = nc.alloc_semaphore(name="wb_dma_sem")

    with tile.TileContext(nc) as tc, ExitStack() as ctx:
        # flat [P, bytes] tiles everywhere: 3D tiles pad the middle free dim
        # to 32 and waste 4x SBUF; views supply the shaped access patterns.
        xpool = ctx.enter_context(tc.tile_pool(name="xin", bufs=6))
        zpool = ctx.enter_context(tc.tile_pool(name="zbuf", bufs=4))
        opools = [ctx.enter_context(tc.tile_pool(name=f"outp{g}", bufs=1))
                  for g in range(len(groups))]
        mpool = ctx.enter_context(tc.tile_pool(name="minip", bufs=1))
        ipool = ctx.enter_context(tc.tile_pool(name="idxp", bufs=1))

        idxs = ipool.tile([P, NG + 1], I32, name="idxs")
        nc.scalar.dma_start(idxs, idxs_dram)

        # staging buffers: one per big group, one shared by the minis
        ots = [opools[g].tile([P, n * RO * OW], F32, name=f"ot{g}")
               for g, (_, n, _) in enumerate(groups)]
        mt = mpool.tile([P, NMINI * (MR // 2) * OW], F32, name="mt")



        wi = 0  # next writeback group index
        for ti, (r0, nr) in enumerate(tiles):
            ro = nr // 2
            xe = xpool.tile([P, ro * W], F32, name="xe")
            xo = xpool.tile([P, ro * W], F32, name="xo")
            nc.sync.dma_start(xe.rearrange("p (r w) -> p r w", w=W),
                              xrows[:, r0 // 2:r0 // 2 + ro, 0, :])
            nc.sync.dma_start(xo.rearrange("p (r w) -> p r w", w=W),
                              xrows[:, r0 // 2:r0 // 2 + ro, 1, :])

            z = zpool.tile([P, ro * W], F32, name="z")
            nc.vector._custom_dve(
                MAXABS_POOL,
                out=z.rearrange("p (s n) -> p s n", n=2),
                in0=xe.rearrange("p (s n) -> p s n", n=2),
                in1=xo.rearrange("p (s n) -> p s n", n=2),
            )

            zc = z.rearrange("p (s n) -> p s n", n=2)[:, :, 1]
            if ti < NT - 1:
                g = wi
                t0, n, q = groups[g]
                off = (ti - t0) * RO * OW
                nc.scalar.tensor_copy(ots[g][:, off:off + RO * OW], zc)
                if ti == t0 + n - 1:
                    nc.gpsimd.kv_writeback(
                        yv,
                        ots[g].rearrange("p (a b q) -> p a b q", a=1, b=1),
                        idxs[:, g:g + 1],
                        queue_num=q,
                    )
                    wi += 1
            else:
                j = ti - (NT - 1)
                mq = MR // 2 * OW
                nc.scalar.tensor_copy(mt[:, j * mq:(j + 1) * mq], zc)
                if j == NMINI - 1:
                    nc.gpsimd.kv_writeback(
                        yv,
                        mt.rearrange("p (a b q) -> p a b q", a=1, b=1),
                        idxs[:, NG:NG + 1],
                        queue_num=3,
                    )

    nc.compile()
    return nc


_nc_cache = []


def kernel(x: np.ndarray) -> np.ndarray:
    x = np.asarray(x, dtype=np.float32)
    assert x.shape == (N, C, H, W)
    if not _nc_cache:
        _nc_cache.append(build_nc())
    nc = _nc_cache[0]

    in_maps = [
        {"x": np.ascontiguousarray(x[k * NB:(k + 1) * NB].reshape(P, H, W))}
        for k in range(NCORES)
    ]
    res = run_bass_kernel_spmd(nc, in_maps, core_ids=list(range(NCORES)))
    out = np.stack([next(iter(r.values())) for r in res.results])
    return out.reshape(N, C, OH, OW)

